# revision 1
# baseline (speedup 1.0000x reference)
"""Trainium2 Bass kernel for modulated deformable conv2d (torchvision semantics).

Problem (hardcoded): input [4,64,128,128] f32, offset [4,18,128,128], mask
[4,9,128,128], weight [64,64,3,3], bias [64]; stride 1, pad 1, dil 1.

Strategy (8 NeuronCores, SPMD, no collectives):
  - Shard: core = (sample b, row-half h).  Each core computes output rows
    [h*64, h*64+64) of sample b => out slice [64, 8192] f32.
  - Bilinear gather is reformulated via difference planes:
        val = I[y0,x0] + lx*D01[y0,x0] + ly*D10[y0,x0] + lx*ly*D11[y0,x0]
    where D01/D10/D11 are x/y/xy forward-difference images on a zero-padded
    grid.  This makes every sample point a SINGLE gather of one 512-byte row
    [I(64c), D01(64c), D10(64c), D11(64c)] in bf16 from an SBUF-resident
    table, fetched with GPSIMD dma_gather (transpose mode) so the gathered
    data lands channels-on-partitions, ready as TensorE rhs.
  - The 4 per-sample scalars (m, m*lx, m*ly, m*lx*ly) are broadcast across
    64 partitions with a tiny contraction-2 PE matmul (block-ones lhsT),
    copied PSUM->SBUF(bf16) on ScalarE, multiplied into the gathered rows on
    VectorE (bf16 2x), and the conv contraction (c,k -> o) + the bilinear
    plane sum run as one accumulated PE matmul chain per 512-pixel block.
"""

import sys

if "/opt/trn_rl_repo" not in sys.path:
    sys.path.insert(0, "/opt/trn_rl_repo")

import numpy as np
import ml_dtypes

BF16 = ml_dtypes.bfloat16

# problem dims
B, C, H, W = 4, 64, 128, 128
O, K = 64, 9
PAD = 8                     # gather window margin (|offset| <= ~6.8 required)
TG = H + 2 * PAD + 1        # 145: table grid covers y,x in [-PAD, H+PAD]
GEXT = TG + 1               # 146: extended image grid (D planes read +1)
NROWS = TG * TG             # 21025
RANKS = (NROWS + 127) // 128  # 165
NROWS_PAD = RANKS * 128     # 21120
NPIX = H * W // 2           # 8192 output pixels per core
NBLK = 16                   # pixel blocks per core
BLK = NPIX // NBLK          # 512 pixels per block
CALL = K * BLK              # 4608 gather indices per block (all 9 taps)
NSAMP = K * NPIX            # 73728 sample points per core
MAGIC = 12582912.0          # 1.5 * 2**23: float32 round-to-int magic

_CACHE = {}


def _split_excess_waits(nc, limit=1):
    """Walrus in this image caps sync-wait commands per instruction; hoist
    excess waits onto preceding same-engine NoOps (engine streams are
    in-order, so blocking earlier on a prefix of the waits is equivalent)."""
    from concourse import mybir

    n = 0
    for fn in nc.m.functions:
        for blk in fn.blocks:
            new = []
            for inst in blk.instructions:
                si = inst.sync_info
                if si is not None and len(si.on_wait) > limit:
                    waits = list(si.on_wait)
                    head, keep = waits[:-limit], waits[-limit:]
                    for i in range(0, len(head), limit):
                        n += 1
                        new.append(mybir.InstNoOp(
                            name=f"waitsplit_{n}",
                            sync_info=mybir.SyncInfo(
                                on_wait=head[i:i + limit], on_update=[]),
                            bass_nofuse=True,
                            engine=inst.engine,
                        ))
                    inst.sync_info = mybir.SyncInfo(
                        on_wait=keep, on_update=list(si.on_update))
                new.append(inst)
            blk.instructions = new


def _build_program():
    import concourse.bass as bass
    import concourse.tile as tile
    from concourse import mybir

    f32 = mybir.dt.float32
    bf16 = mybir.dt.bfloat16
    i16 = mybir.dt.int16
    ADD = mybir.AluOpType.add
    MULT = mybir.AluOpType.mult

    nc = bass.Bass("TRN2", target_bir_lowering=False, debug=False,
                   enable_asserts=False, dynamic_dma_scratch_size=65536)

    tab_d = nc.dram_tensor("tab", [128, RANKS, 4 * C], bf16, kind="ExternalInput")
    oy1_d = nc.dram_tensor("oy1", [128, NSAMP // 128], f32, kind="ExternalInput")
    ox1_d = nc.dram_tensor("ox1", [128, NSAMP // 128], f32, kind="ExternalInput")
    m1_d = nc.dram_tensor("m1", [128, NSAMP // 128], f32, kind="ExternalInput")
    oy2_d = nc.dram_tensor("oy2", [16, NSAMP // 16], f32, kind="ExternalInput")
    ox2_d = nc.dram_tensor("ox2", [16, NSAMP // 16], f32, kind="ExternalInput")
    ci2_d = nc.dram_tensor("ci2", [16, NSAMP // 16], f32, kind="ExternalInput")
    wt_d = nc.dram_tensor("wt", [C, K * O], f32, kind="ExternalInput")
    ones2_d = nc.dram_tensor("ones2", [2, 128], f32, kind="ExternalInput")
    bias_d = nc.dram_tensor("bias", [O, 1], f32, kind="ExternalInput")
    out_d = nc.dram_tensor("out", [O, NPIX], f32, kind="ExternalOutput")

    L1F = NSAMP // 128      # 576
    L2F = NSAMP // 16       # 4608

    from concourse import library_config

    with tile.TileContext(nc) as tc:
        nc.gpsimd.load_library(library_config.mlp)   # provides DMAGatherAnt
        with tc.tile_pool(name="const", bufs=1) as cp:
            tab = cp.tile([128, RANKS, 4 * C], bf16, tag="tab")
            nc.sync.dma_start(tab[:], tab_d.ap())

            w_sb = cp.tile([128, K * O], bf16, tag="wsb")
            nc.gpsimd.dma_start(w_sb[0:64, :], wt_d.ap())      # cast f32->bf16
            nc.gpsimd.dma_start(w_sb[64:128, :], wt_d.ap())

            bias_sb = cp.tile([O, 1], f32, tag="bias")
            nc.sync.dma_start(bias_sb[:], bias_d.ap())

            ones2 = cp.tile([2, 128], bf16, tag="ones2")
            nc.gpsimd.dma_start(ones2[:], ones2_d.ap())   # cast f32->bf16

            idxr = cp.tile([128, L2F], i16, tag="idxr")
            # beta planes, wrapped: bw[q = t*8+sub, plane, e] = beta_plane(j),
            # j = q*576 + e = t*4608 + (sub*576 + e)
            bw = cp.tile([128, 4, L1F], bf16, tag="bw")

            # ---------- prep: gather indices (16-wrapped layout) ----------
            with tc.tile_pool(name="prep", bufs=1) as pp:
                for hf in range(2):
                    sl = slice(hf * (L2F // 2), (hf + 1) * (L2F // 2))
                    a = pp.tile([16, L2F // 2], f32, tag="p2a")
                    bt = pp.tile([16, L2F // 2], f32, tag="p2b")
                    nc.sync.dma_start(a[:], oy2_d.ap()[:, sl])
                    # floor(v) = round(v - 0.5) for non-integer v
                    nc.vector.tensor_scalar(a[:], a[:], -0.5, MAGIC, ADD, ADD)
                    nc.vector.tensor_scalar(a[:], a[:], -MAGIC, float(TG), ADD, MULT)
                    nc.sync.dma_start(bt[:], ox2_d.ap()[:, sl])
                    nc.vector.tensor_scalar(bt[:], bt[:], -0.5, MAGIC, ADD, ADD)
                    nc.vector.tensor_scalar(bt[:], bt[:], -MAGIC, None, ADD)
                    nc.vector.tensor_add(a[:], a[:], bt[:])
                    nc.sync.dma_start(bt[:], ci2_d.ap()[:, sl])
                    nc.vector.tensor_add(a[:], a[:], bt[:])
                    nc.vector.tensor_copy(idxr[0:16, sl], a[:])  # cast -> int16
                # replicate the 16-partition wrap to all 8 core groups
                for g in range(1, 8):
                    nc.sync.dma_start(idxr[16 * g:16 * (g + 1), :], idxr[0:16, :])

                # ---------- prep: beta planes (m, m*lx, m*ly, m*lx*ly) ----------
                v = pp.tile([128, L1F], f32, tag="p1a")
                fl = pp.tile([128, L1F], f32, tag="p1b")
                lx = pp.tile([128, L1F], f32, tag="p1c")
                ly = pp.tile([128, L1F], f32, tag="p1d")
                mm = pp.tile([128, L1F], f32, tag="p1e")

                nc.sync.dma_start(v[:], oy1_d.ap())
                nc.vector.tensor_scalar(fl[:], v[:], -0.5, MAGIC, ADD, ADD)
                nc.vector.tensor_scalar(fl[:], fl[:], -MAGIC, None, ADD)
                nc.vector.tensor_sub(ly[:], v[:], fl[:])
                nc.sync.dma_start(v[:], ox1_d.ap())
                nc.vector.tensor_scalar(fl[:], v[:], -0.5, MAGIC, ADD, ADD)
                nc.vector.tensor_scalar(fl[:], fl[:], -MAGIC, None, ADD)
                nc.vector.tensor_sub(lx[:], v[:], fl[:])
                nc.sync.dma_start(mm[:], m1_d.ap())

                nc.vector.tensor_copy(bw[:, 0, :], mm[:])            # m
                nc.vector.tensor_mul(v[:], mm[:], lx[:])
                nc.vector.tensor_copy(bw[:, 1, :], v[:])             # m*lx
                nc.vector.tensor_mul(fl[:], mm[:], ly[:])
                nc.vector.tensor_copy(bw[:, 2, :], fl[:])            # m*ly
                nc.vector.tensor_mul(v[:], fl[:], lx[:])
                nc.vector.tensor_copy(bw[:, 3, :], v[:])             # m*lx*ly

            # ---------- main loop over 16 pixel blocks ----------
            GK = 3                       # taps per gather call
            GCALL = GK * BLK             # gather indices per call
            nidx_reg = nc.gpsimd.to_reg(GK * BLK)
            with (
                tc.tile_pool(name="g", bufs=3) as gp,
                tc.tile_pool(name="bst", bufs=1) as bstp,
                tc.tile_pool(name="bpsum", bufs=4, space="PSUM") as bpp,
                tc.tile_pool(name="opsum", bufs=2, space="PSUM") as opp,
                tc.tile_pool(name="val", bufs=4) as vp,
                tc.tile_pool(name="bscp", bufs=4) as bsp,
                tc.tile_pool(name="ob", bufs=2) as obp,
            ):
                for t in range(NBLK):
                    gs = []
                    for kg in range(K // GK):
                        g = gp.tile([128, 2, GCALL], bf16, tag="g")
                        s0 = t * (CALL // 16) + kg * (GCALL // 16)
                        nc.gpsimd.dma_gather(
                            g[:],
                            tab[:],
                            idxr[:, s0:s0 + GCALL // 16],
                            GCALL,
                            nidx_reg,
                            4 * C,          # elem_size in bf16 units = 512 B
                            transpose=True,
                            single_packet=False,
                            sbuf_tokens_per_rank=128,
                            sbuf_free_dim_per_rank=512,
                        )
                        gs.append(g)
                    # stage this block's beta rows at partitions 0-1 for PE rhs
                    bstA = bstp.tile([2, CALL], bf16, tag="bstA")
                    bstB = bstp.tile([2, CALL], bf16, tag="bstB")
                    src = bw[8 * t:8 * (t + 1), :, :]
                    nc.sync.dma_start(bstA[0:1, :], src[:, 0, :])
                    nc.sync.dma_start(bstA[1:2, :], src[:, 1, :])
                    nc.sync.dma_start(bstB[0:1, :], src[:, 2, :])
                    nc.sync.dma_start(bstB[1:2, :], src[:, 3, :])

                    ops = opp.tile([O, BLK], f32, tag="ops")
                    for k in range(K):
                        sl = slice(k * BLK, (k + 1) * BLK)
                        gsl = slice((k % GK) * BLK, (k % GK + 1) * BLK)
                        g = gs[k // GK]
                        bpA = bpp.tile([128, BLK], f32, tag="bp")
                        nc.tensor.matmul(bpA[:], ones2[:], bstA[:, sl],
                                         start=True, stop=True)
                        bpB = bpp.tile([128, BLK], f32, tag="bp")
                        nc.tensor.matmul(bpB[:], ones2[:], bstB[:, sl],
                                         start=True, stop=True)
                        bsA = bsp.tile([128, BLK], bf16, tag="bsc")
                        nc.scalar.copy(bsA[:], bpA[:])
                        bsB = bsp.tile([128, BLK], bf16, tag="bsc")
                        nc.scalar.copy(bsB[:], bpB[:])
                        vA = vp.tile([128, BLK], bf16, tag="v")
                        nc.vector.tensor_mul(vA[:], g[:, 0, gsl], bsA[:])
                        vB = vp.tile([128, BLK], bf16, tag="v")
                        nc.vector.tensor_mul(vB[:], g[:, 1, gsl], bsB[:])
                        wk = w_sb[:, k * O:(k + 1) * O]
                        nc.tensor.matmul(ops[:], wk, vA[:],
                                         start=(k == 0), stop=False)
                        nc.tensor.matmul(ops[:], wk, vB[:],
                                         start=False, stop=(k == K - 1))

                    ob = obp.tile([O, BLK], f32, tag="ob")
                    nc.scalar.add(ob[:], ops[:], bias_sb[:, 0:1])
                    nc.sync.dma_start(out_d.ap()[:, t * BLK:(t + 1) * BLK], ob[:])

    _split_excess_waits(nc)
    # populate .instr bytes of extended-inst InstISA subclasses (DMAGatherAnt,
    # PseudoReloadLibraryIndex) — Bacc does this in compile(); raw Bass must
    # call it explicitly or walrus fails with "ISA wrong length".
    from concourse.library_overlay import lower_extended_insts
    lower_extended_insts(nc)
    return nc


def _host_prep(input, offset, mask, weight, bias):
    x = np.asarray(input, np.float32)
    off = np.asarray(offset, np.float32)
    msk = np.asarray(mask, np.float32)
    w = np.asarray(weight, np.float32)
    b = np.asarray(bias, np.float32)

    amax = float(np.abs(off).max())
    if amax >= PAD - 1.2:
        raise ValueError(f"offset magnitude {amax} exceeds supported window")

    f32 = np.float32

    # per-sample gather tables
    tabs = []
    for bb in range(B):
        E = np.zeros((C, GEXT, GEXT), f32)
        E[:, PAD:PAD + H, PAD:PAD + W] = x[bb]
        Eb = E.astype(BF16).astype(f32)
        D01 = np.zeros((C, GEXT, GEXT), f32)
        D01[:, :, :-1] = Eb[:, :, 1:] - Eb[:, :, :-1]
        D10 = np.zeros((C, GEXT, GEXT), f32)
        D10[:, :-1, :] = Eb[:, 1:, :] - Eb[:, :-1, :]
        D11 = np.zeros((C, GEXT, GEXT), f32)
        D11[:, :-1, :-1] = (Eb[:, 1:, 1:] - Eb[:, 1:, :-1]
                            - Eb[:, :-1, 1:] + Eb[:, :-1, :-1])
        planes = np.stack([Eb, D01, D10, D11], 0)[:, :, :TG, :TG]  # [4,C,TG,TG]
        rows = np.zeros((NROWS_PAD, 4, C), BF16)
        rows[:NROWS] = planes.transpose(2, 3, 0, 1).reshape(NROWS, 4, C).astype(BF16)
        tabdram = np.ascontiguousarray(
            rows.reshape(RANKS, 128, 4 * C).transpose(1, 0, 2))   # [128,165,256]
        tabs.append(tabdram)

    wt = np.ascontiguousarray(
        w.reshape(O, C, K).transpose(1, 2, 0).reshape(C, K * O)).astype(f32)
    ones2c = np.zeros((2, 128), f32)
    ones2c[0, 0:64] = 1.0
    ones2c[1, 64:128] = 1.0
    bias2 = np.ascontiguousarray(b.reshape(O, 1))

    karr = np.arange(K)
    p = np.arange(NPIX)
    ylo = p // W
    xloc = p % W

    def tojd(a):  # [K, NPIX] -> flat j order (t, k, ptilde)
        return np.ascontiguousarray(
            a.reshape(K, NBLK, BLK).transpose(1, 0, 2).reshape(-1).astype(f32))

    in_maps = []
    for core in range(8):
        bb, h = divmod(core, 2)
        yg = h * 64 + ylo                                   # [NPIX] global y
        offv = off[bb].reshape(K, 2, H, W)
        oy_kp = offv[:, 0][:, yg, xloc]                     # [K, NPIX]
        ox_kp = offv[:, 1][:, yg, xloc]
        m_kp = msk[bb][:, yg, xloc]
        by = yg[None, :] - 1 + (karr // 3)[:, None]
        bx = xloc[None, :] - 1 + (karr % 3)[:, None]
        ci_kp = ((by + PAD) * TG + (bx + PAD)).astype(f32)

        oyj, oxj, mj, cij = tojd(oy_kp), tojd(ox_kp), tojd(m_kp), tojd(ci_kp)
        l2 = lambda a: np.ascontiguousarray(a.reshape(NSAMP // 16, 16).T)
        in_maps.append({
            "tab": tabs[bb],
            "oy1": oyj.reshape(128, NSAMP // 128),
            "ox1": oxj.reshape(128, NSAMP // 128),
            "m1": mj.reshape(128, NSAMP // 128),
            "oy2": l2(oyj),
            "ox2": l2(oxj),
            "ci2": l2(cij),
            "wt": wt,
            "ones2": ones2c,
            "bias": bias2,
        })
    return in_maps


def _install_ntff_shim():
    """Provide antenv.axon_hooks (missing in this image) so trace=True works."""
    import types
    if "antenv.axon_hooks" in sys.modules:
        return
    sys.path.insert(0, "/root/.axon_site")
    from trn_agent_boot.trn_boot import _ntff_profile_via_ctypes
    hook = _ntff_profile_via_ctypes("/opt/axon/libaxon_pjrt.so")
    mod = types.ModuleType("antenv.axon_hooks")
    mod.get_axon_ntff_profile_hook = lambda: hook
    mod.set_axon_ntff_profile_hook = lambda h: None
    sys.modules["antenv.axon_hooks"] = mod


def kernel(input, offset, mask, weight, bias, _trace=False):
    if _trace:
        _install_ntff_shim()
    from concourse.bass_utils import run_bass_kernel_spmd

    if "nc" not in _CACHE:
        _CACHE["nc"] = _build_program()
    nc = _CACHE["nc"]

    in_maps = _host_prep(input, offset, mask, weight, bias)
    res = run_bass_kernel_spmd(
        nc, in_maps, core_ids=list(range(8)),
        trace=_trace,
        trace_cores=list(range(8)) if _trace else None,
    )
    kernel.last_results = res

    out = np.empty((B, O, H, W), np.float32)
    for core in range(8):
        bb, h = divmod(core, 2)
        blockout = res.results[core]["out"]       # [64, 8192] f32
        out[bb, :, h * 64:(h + 1) * 64, :] = blockout.reshape(O, 64, W)
    return out



# revision 12
# speedup vs baseline: 1.6558x; 1.6558x over previous
"""Trainium2 Bass kernel for modulated deformable conv2d (torchvision semantics).

Problem (hardcoded): input [4,64,128,128] f32, offset [4,18,128,128], mask
[4,9,128,128], weight [64,64,3,3], bias [64]; stride 1, pad 1, dil 1.

Strategy (8 NeuronCores, SPMD, no collectives):
  - Shard: core = (sample b, row-half h).  Each core computes output rows
    [h*64, h*64+64) of sample b => out slice [64, 8192] f32.
  - Bilinear gather via difference planes:
        val = I[y0,x0] + lx*D01[y0,x0] + ly*D10[y0,x0] + lx*ly*D11[y0,x0]
    so each sample point is ONE 512-byte gather [I(64c),D01,D10,D11] bf16
    from an SBUF table via GPSIMD dma_gather (transpose mode), landing
    channels-on-partitions.
  - Gather descriptor generation is round-robined over 4 SWDGE queues so all
    8 Q7 cores (4 pairs) generate descriptors concurrently.
  - Gather indices and the per-sample betas (m, m*lx, m*ly, m*lx*ly) are
    host-precomputed; betas ship as 2-partition row pairs and are broadcast
    across 64 partitions on the PE with a constant row-selector lhsT.
  - PE runs entirely in 64x128 tiling mode: tile T0 (SBUF rows 0-63) and T8
    (rows 64-127) execute beta-broadcast and conv matmuls concurrently.
    Conv lhsT is [w | 0] zero-padded to 128 output columns so every matmul
    has tile_size (64,128); accumulators accI (I/D01 planes) and accD
    (D10/D11) merge + bias at block end.
  - beta*g products: ~3/4 via ScalarE psum->sbuf bf16 copy + VectorE 2x bf16
    mul, ~1/4 via VectorE psum-direct mul, balancing both engines.
"""

import sys

if "/opt/trn_rl_repo" not in sys.path:
    sys.path.insert(0, "/opt/trn_rl_repo")

import numpy as np
import ml_dtypes

BF16 = ml_dtypes.bfloat16

# problem dims
B, C, H, W = 4, 64, 128, 128
O, K = 64, 9
PAD = 8                     # gather window margin (|offset| <= ~6.8 required)
TG = H + 2 * PAD + 1        # 145: table grid covers y,x in [-PAD, H+PAD]
GEXT = TG + 1               # 146: extended image grid (D planes read +1)
NROWS = TG * TG             # 21025
RANKS = (NROWS + 127) // 128  # 165
NROWS_PAD = RANKS * 128     # 21120
NPIX = H * W // 2           # 8192 output pixels per core
NBLK = 16                   # pixel blocks per core
BLK = NPIX // NBLK          # 512 pixels per block
NSAMP = K * NPIX            # 73728 sample points per core
GK = 3                      # taps per gather call
GCALL = GK * BLK            # 1536 gather indices per call
NCALL = NBLK * (K // GK)    # 48 gather calls per core
L2F = NSAMP // 16           # 4608 idx columns (16-wrapped)
NQ = 4                      # SWDGE queues
BSTB = 3                    # beta staging buffers

_CACHE = {}


def _split_excess_waits(nc, limit=1):
    """Walrus in this image caps sync-wait commands per instruction; hoist
    excess waits onto preceding same-engine NoOps (engine streams are
    in-order, so blocking earlier on a prefix of the waits is equivalent)."""
    from concourse import mybir

    n = 0
    for fn in nc.m.functions:
        for blk in fn.blocks:
            new = []
            for inst in blk.instructions:
                si = inst.sync_info
                if si is not None and len(si.on_wait) > limit:
                    waits = list(si.on_wait)
                    head, keep = waits[:-limit], waits[-limit:]
                    for i in range(0, len(head), limit):
                        n += 1
                        new.append(mybir.InstNoOp(
                            name=f"waitsplit_{n}",
                            sync_info=mybir.SyncInfo(
                                on_wait=head[i:i + limit], on_update=[]),
                            bass_nofuse=True,
                            engine=inst.engine,
                        ))
                    inst.sync_info = mybir.SyncInfo(
                        on_wait=keep, on_update=list(si.on_update))
                new.append(inst)
            blk.instructions = new


def _build_program():
    import concourse.bass as bass
    import concourse.tile as tile
    from concourse import mybir

    f32 = mybir.dt.float32
    bf16 = mybir.dt.bfloat16
    i16 = mybir.dt.int16

    nc = bass.Bass("TRN2", target_bir_lowering=False, debug=False,
                   enable_asserts=False, dynamic_dma_scratch_size=57344,
                   num_swdge_queues=NQ)

    tab_d = nc.dram_tensor("tab", [128, RANKS, 4 * C], bf16, kind="ExternalInput")
    idxr_d = nc.dram_tensor("idxr", [128, L2F], i16, kind="ExternalInput")
    ba_d = nc.dram_tensor("ba", [NCALL, 2, GCALL], bf16, kind="ExternalInput")
    bb_d = nc.dram_tensor("bb", [NCALL, 2, GCALL], bf16, kind="ExternalInput")
    wpad_d = nc.dram_tensor("wpad", [128, K, 128], bf16, kind="ExternalInput")
    sel_d = nc.dram_tensor("sel", [128, 128], bf16, kind="ExternalInput")
    bias_d = nc.dram_tensor("bias", [O, 1], f32, kind="ExternalInput")
    out_d = nc.dram_tensor("out", [O, NPIX], f32, kind="ExternalOutput")

    from concourse import library_config

    with tile.TileContext(nc) as tc:
        nc.gpsimd.load_library(library_config.mlp)   # provides DMAGatherAnt
        with tc.tile_pool(name="const", bufs=1) as cp:
            tab = cp.tile([128, RANKS, 4 * C], bf16, tag="tab")
            nc.sync.dma_start(tab[:], tab_d.ap())

            idxr = cp.tile([128, L2F], i16, tag="idxr")
            nc.sync.dma_start(idxr[:], idxr_d.ap())

            wpad = cp.tile([128, K, 128], bf16, tag="wpad")
            nc.sync.dma_start(wpad[:], wpad_d.ap())

            sel = cp.tile([128, 128], bf16, tag="sel")
            nc.sync.dma_start(sel[:], sel_d.ap())

            bias_sb = cp.tile([O, 1], f32, tag="bias")
            nc.sync.dma_start(bias_sb[:], bias_d.ap())

            # beta staging: rows 0-1 = (m, m*lx) for T0, rows 64-65 =
            # (m*ly, m*lx*ly) for T8; all other partitions stay zero so the
            # zero rows of the selector lhsT never multiply NaN garbage.
            bst = cp.tile([128, BSTB, GCALL], bf16, tag="bst")
            nc.vector.memset(bst[:], 0.0)

            nidx_reg = nc.gpsimd.to_reg(GCALL)

            with (
                tc.tile_pool(name="g", bufs=3) as gp,
                tc.tile_pool(name="bp", bufs=4, space="PSUM") as bpp,
                tc.tile_pool(name="accI", bufs=2, space="PSUM") as aip,
                tc.tile_pool(name="accD", bufs=2, space="PSUM") as adp,
                tc.tile_pool(name="v", bufs=4) as vp,
                tc.tile_pool(name="bs", bufs=4) as bsp,
                tc.tile_pool(name="mg", bufs=2) as mgp,
                tc.tile_pool(name="ob", bufs=2) as obp,
            ):
                # Block 0 is computed twice: the first pass (index NBLK,
                # discarded) absorbs a hardware-only startup corruption of
                # the first few matmuls (PE tiling-mode config lands after
                # the first ldweights); the redo at the end runs on a warm,
                # correctly-configured array and produces the real output.
                for it in range(NBLK + 1):
                    warm = it == 0
                    t = 0 if warm else it - 1
                    accI = aip.tile([128, BLK], f32, tag="accI")
                    accD = adp.tile([128, BLK], f32, tag="accD")
                    gs = []
                    for kg in range(K // GK):
                        c = t * (K // GK) + kg
                        g = gp.tile([128, 2, GCALL], bf16, tag="g")
                        nc.gpsimd.dma_gather(
                            g[:],
                            tab[:],
                            idxr[:, c * (GCALL // 16):(c + 1) * (GCALL // 16)],
                            GCALL,
                            nidx_reg,
                            4 * C,          # elem_size bf16 units = 512 B
                            transpose=True,
                            single_packet=False,
                            sbuf_tokens_per_rank=128,
                            sbuf_free_dim_per_rank=512,
                            queue_num=c % NQ,
                        )
                        gs.append(g)
                        bb = c % BSTB
                        nc.sync.dma_start(bst[0:2, bb, :], ba_d.ap()[c])
                        nc.sync.dma_start(bst[64:66, bb, :], bb_d.ap()[c])

                    # software-pipelined: betas one tap ahead of main matmuls
                    bps = {}
                    for k in range(K + 1):
                        if k < K:
                            c = t * (K // GK) + k // GK
                            bb = c % BSTB
                            sl = slice((k % GK) * BLK, (k % GK + 1) * BLK)
                            bp0 = bpp.tile([128, BLK], f32, tag="bp")
                            nc.tensor.matmul(bp0[:], sel[0:64, :],
                                             bst[0:64, bb, sl],
                                             start=True, stop=True)
                            bp1 = bpp.tile([128, BLK], f32, tag="bp")
                            nc.tensor.matmul(bp1[:], sel[64:128, :],
                                             bst[64:128, bb, sl],
                                             start=True, stop=True)
                            g = gs[k // GK]
                            vs = []
                            for s in range(2):
                                bp = (bp0, bp1)[s]
                                v = vp.tile([128, BLK], bf16, tag="v")
                                midx = (t * K + k) * 2 + s
                                if midx % 4 == 3:
                                    # VectorE psum-direct (1x) path
                                    nc.vector.tensor_mul(v[:], g[:, s, sl], bp[:])
                                else:
                                    # ScalarE copy + VectorE 2x bf16 path
                                    bsc = bsp.tile([128, BLK], bf16, tag="bs")
                                    nc.scalar.copy(bsc[:], bp[:])
                                    nc.vector.tensor_mul(v[:], g[:, s, sl], bsc[:])
                                vs.append(v)
                            bps[k] = vs
                        if k > 0:
                            km = k - 1
                            vs = bps.pop(km)
                            wsl = wpad[:, km, :]
                            for s in range(2):
                                v = vs[s]
                                first = (km == 0 and s == 0)
                                last = (km == K - 1 and s == 1)
                                nc.tensor.matmul(accI[:], wsl[0:64, :],
                                                 v[0:64, :],
                                                 start=first, stop=last)
                                nc.tensor.matmul(accD[:], wsl[64:128, :],
                                                 v[64:128, :],
                                                 start=first, stop=last)

                    mg = mgp.tile([O, BLK], f32, tag="mg")
                    nc.scalar.add(mg[:], accD[0:64, :], bias_sb[:, 0:1])
                    ob = obp.tile([O, BLK], f32, tag="ob")
                    nc.vector.tensor_add(ob[:], accI[0:64, :], mg[:])
                    if not warm:
                        nc.sync.dma_start(out_d.ap()[:, t * BLK:(t + 1) * BLK],
                                          ob[:])

    _split_excess_waits(nc)
    # populate .instr bytes of extended-inst InstISA subclasses (DMAGatherAnt,
    # PseudoReloadLibraryIndex) — Bacc does this in compile(); raw Bass must
    # call it explicitly or walrus fails with "ISA wrong length".
    from concourse.library_overlay import lower_extended_insts
    lower_extended_insts(nc)
    return nc


def _host_prep(input, offset, mask, weight, bias):
    x = np.asarray(input, np.float32)
    off = np.asarray(offset, np.float32)
    msk = np.asarray(mask, np.float32)
    w = np.asarray(weight, np.float32)
    b = np.asarray(bias, np.float32)

    amax = float(np.abs(off).max())
    if amax >= PAD - 1.2:
        raise ValueError(f"offset magnitude {amax} exceeds supported window")

    f32 = np.float32

    # per-sample gather tables
    tabs = []
    for bb in range(B):
        E = np.zeros((C, GEXT, GEXT), f32)
        E[:, PAD:PAD + H, PAD:PAD + W] = x[bb]
        Eb = E.astype(BF16).astype(f32)
        D01 = np.zeros((C, GEXT, GEXT), f32)
        D01[:, :, :-1] = Eb[:, :, 1:] - Eb[:, :, :-1]
        D10 = np.zeros((C, GEXT, GEXT), f32)
        D10[:, :-1, :] = Eb[:, 1:, :] - Eb[:, :-1, :]
        D11 = np.zeros((C, GEXT, GEXT), f32)
        D11[:, :-1, :-1] = (Eb[:, 1:, 1:] - Eb[:, 1:, :-1]
                            - Eb[:, :-1, 1:] + Eb[:, :-1, :-1])
        planes = np.stack([Eb, D01, D10, D11], 0)[:, :, :TG, :TG]  # [4,C,TG,TG]
        rows = np.zeros((NROWS_PAD, 4, C), BF16)
        rows[:NROWS] = planes.transpose(2, 3, 0, 1).reshape(NROWS, 4, C).astype(BF16)
        tabdram = np.ascontiguousarray(
            rows.reshape(RANKS, 128, 4 * C).transpose(1, 0, 2))   # [128,165,256]
        tabs.append(tabdram)

    # conv weights: [c, k, o] zero-padded to 128 output cols, both halves
    wck = w.reshape(O, C, K).transpose(1, 2, 0)        # [C, K, O]
    wpad = np.zeros((128, K, 128), f32)
    wpad[0:64, :, 0:64] = wck
    wpad[64:128, :, 0:64] = wck
    wpad = wpad.astype(BF16)

    # beta-broadcast row selector: out row block <- rhs partition row {0,1}
    selm = np.zeros((128, 128), f32)
    selm[0, 0:64] = 1.0
    selm[1, 64:128] = 1.0
    selm[64, 0:64] = 1.0
    selm[65, 64:128] = 1.0
    selm = selm.astype(BF16)

    bias2 = np.ascontiguousarray(b.reshape(O, 1))

    karr = np.arange(K)
    p = np.arange(NPIX)
    ylo = p // W
    xloc = p % W

    def tojd(a):  # [K, NPIX] -> flat j order (t, k, ptilde)
        return np.ascontiguousarray(
            a.reshape(K, NBLK, BLK).transpose(1, 0, 2).reshape(-1).astype(f32))

    in_maps = []
    for core in range(8):
        bb, h = divmod(core, 2)
        yg = h * 64 + ylo                                   # [NPIX] global y
        offv = off[bb].reshape(K, 2, H, W)
        oy_kp = offv[:, 0][:, yg, xloc]                     # [K, NPIX]
        ox_kp = offv[:, 1][:, yg, xloc]
        m_kp = msk[bb][:, yg, xloc]
        by = yg[None, :] - 1 + (karr // 3)[:, None]
        bx = xloc[None, :] - 1 + (karr % 3)[:, None]

        fy = np.floor(oy_kp)
        fx = np.floor(ox_kp)
        ly = oy_kp - fy
        lx = ox_kp - fx
        idx = ((fy + by + PAD) * TG + (fx + bx + PAD))      # [K, NPIX] float

        idxj = tojd(idx)
        mj = tojd(m_kp)
        lxj = tojd(lx)
        lyj = tojd(ly)

        idxr = np.ascontiguousarray(
            idxj.reshape(L2F, 16).T.astype(np.int16))       # [16, L2F]
        idxr = np.tile(idxr, (8, 1))                        # [128, L2F]

        ba = np.stack([mj, mj * lxj])                       # [2, NSAMP]
        bbta = np.stack([mj * lyj, mj * lyj * lxj])
        ba = np.ascontiguousarray(
            ba.reshape(2, NCALL, GCALL).transpose(1, 0, 2)).astype(BF16)
        bbta = np.ascontiguousarray(
            bbta.reshape(2, NCALL, GCALL).transpose(1, 0, 2)).astype(BF16)

        in_maps.append({
            "tab": tabs[bb],
            "idxr": idxr,
            "ba": ba,
            "bb": bbta,
            "wpad": wpad,
            "sel": selm,
            "bias": bias2,
        })
    return in_maps


def _install_ntff_shim():
    """Provide antenv.axon_hooks (missing in this image) so trace=True works."""
    import types
    if "antenv.axon_hooks" in sys.modules:
        return
    sys.path.insert(0, "/root/.axon_site")
    from trn_agent_boot.trn_boot import _ntff_profile_via_ctypes
    hook = _ntff_profile_via_ctypes("/opt/axon/libaxon_pjrt.so")
    mod = types.ModuleType("antenv.axon_hooks")
    mod.get_axon_ntff_profile_hook = lambda: hook
    mod.set_axon_ntff_profile_hook = lambda h: None
    sys.modules["antenv.axon_hooks"] = mod


def kernel(input, offset, mask, weight, bias, _trace=False):
    if _trace:
        _install_ntff_shim()
    from concourse.bass_utils import run_bass_kernel_spmd

    if "nc" not in _CACHE:
        _CACHE["nc"] = _build_program()
    nc = _CACHE["nc"]

    in_maps = _host_prep(input, offset, mask, weight, bias)
    res = run_bass_kernel_spmd(
        nc, in_maps, core_ids=list(range(8)),
        trace=_trace,
        trace_cores=list(range(8)) if _trace else None,
    )
    kernel.last_results = res

    out = np.empty((B, O, H, W), np.float32)
    for core in range(8):
        bb, h = divmod(core, 2)
        blockout = res.results[core]["out"]       # [64, 8192] f32
        out[bb, :, h * 64:(h + 1) * 64, :] = blockout.reshape(O, 64, W)
    return out
